# revision 1
# baseline (speedup 1.0000x reference)
"""Two-layer GCN (PyG GCNConv x2 + ReLU) on 8 Trainium2 NeuronCores.

Strategy (dst-sharded, SPMD single program):
  - Nodes padded to 102400, sharded 12800/core by destination.
  - Per layer: dense h = x_shard @ W on each core -> AllGather full h ->
    windowed dma_gather of h[src] per edge (4 windows of 25600 rows so the
    int16 gather indices fit) -> scatter-add via per-tile selection-matrix
    matmuls (S[e, dst_local] = norm_e, built on host) accumulated in PSUM
    -> SBUF accumulator -> ReLU epilogue.
  - Layer 1 runs "transposed" (psum[f, d] via lhsT=msg) so its output is
    directly the lhsT operand for layer 2's dense matmul; layer 2 runs
    normal (psum[d, f]) so the final output is row-major node x feature.
  - Symmetric norm a[src]*a[dst] is folded into S; biases: b1 is applied as
    the ACT per-partition bias in the transposed world; b2 via a broadcast
    add only when nonzero (it is zero in this problem's spec).
"""

import numpy as np

import concourse.bass as bass
import concourse.bacc as bacc
import concourse.mybir as mybir
import concourse.tile as tile
from concourse.bass_utils import run_bass_kernel_spmd

N = 100000
E = 640000
D = 128
NCORES = 8
NPAD = 102400
SHARD = NPAD // NCORES        # 12800
NBLK = SHARD // 128           # 100 dst blocks per core
WIN = 25600                   # gather window rows (int16-safe)
NW = NPAD // WIN              # 4 windows
CHUNK_T = 8                   # tiles (of 128 edges) per dma_gather call (1024 idx: ring limit)

_CACHE = {}


def _host_prep(x, edge_index, W1, b1, W2, b2):
    x = np.asarray(x, dtype=np.float32)
    ei = np.asarray(edge_index)
    W1 = np.asarray(W1, dtype=np.float32)
    W2 = np.asarray(W2, dtype=np.float32)
    b1 = np.asarray(b1, dtype=np.float32)
    b2 = np.asarray(b2, dtype=np.float32)
    n = x.shape[0]

    src = np.concatenate([ei[0], np.arange(n, dtype=np.int64)])
    dst = np.concatenate([ei[1], np.arange(n, dtype=np.int64)])
    deg = np.bincount(dst, minlength=NPAD).astype(np.float32)
    a = np.zeros(NPAD, np.float32)
    nz = deg > 0
    a[nz] = 1.0 / np.sqrt(deg[nz])

    # degree-balanced node->position permutation: deal nodes (sorted by degree
    # desc) round-robin over the 800 (core, block) pairs so every block has a
    # near-equal edge count; all device-side structures live in position space.
    order_by_deg = np.argsort(-deg, kind="stable")
    i = np.arange(NPAD, dtype=np.int64)
    cb = i % (NCORES * NBLK)
    position_of_rank = (cb % NCORES) * SHARD + (cb // NCORES) * 128 + i // (NCORES * NBLK)
    pos_of_node = np.empty(NPAD, np.int64)
    pos_of_node[order_by_deg] = position_of_rank
    node_at_pos = np.empty(NPAD, np.int64)
    node_at_pos[pos_of_node] = np.arange(NPAD, dtype=np.int64)

    ps = pos_of_node[src]
    pd = pos_of_node[dst]
    core = pd // SHARD
    # logical gather window per edge: 0 = the appended self-loop window
    # (served from the core-local dense output, so int16-indexable and free of
    # the clustering the node permutation would otherwise cause), 1..NW = the
    # four 25600-row slices of the AllGathered table.
    is_self = np.zeros(src.shape[0], bool)
    is_self[E if src.shape[0] == E + n else src.shape[0] - n:] = True
    NWG = NW + 1

    per_core = []
    counts_all = np.zeros((NCORES, NWG * NBLK), np.int64)
    for k in range(NCORES):
        m = core == k
        s_k = ps[m]
        d_k = pd[m]
        n_s = src[m]
        n_d = dst[m]
        self_k = is_self[m]
        w_k = np.where(self_k, 0, 1 + s_k // WIN)
        b_k = (d_k % SHARD) // 128
        key = w_k * NBLK + b_k
        order = np.lexsort((s_k, key))
        s_k, d_k, key = s_k[order], d_k[order], key[order]
        n_s, n_d, self_k = n_s[order], n_d[order], self_k[order]
        counts = np.bincount(key, minlength=NWG * NBLK)
        counts_all[k] = counts
        per_core.append((s_k, d_k, n_s, n_d, self_k, key, counts))

    # common tile schedule: T[w*NBLK+b] tiles of 128 edges, identical on all cores
    T = (np.max(counts_all, axis=0) + 127) // 128
    T[:NBLK] = np.maximum(T[:NBLK], 1)  # self window groups init the accumulator
    tile_base = np.zeros(NWG * NBLK + 1, np.int64)
    tile_base[1:] = np.cumsum(T)
    t_total = int(tile_base[-1])

    # gather-call schedule: chunks of <= CHUNK_T tiles, never crossing windows
    calls = []  # (window, tile_start, n_tiles); window 0 = local/self
    for w in range(NWG):
        w_start = int(tile_base[w * NBLK])
        w_end = int(tile_base[(w + 1) * NBLK])
        t = w_start
        while t < w_end:
            nt = min(CHUNK_T, w_end - t)
            calls.append((w, t, nt))
            t += nt

    # per-core padded flat arrays in tile order
    in_maps = []
    x_pad = np.zeros((NPAD, D), np.float32)
    x_pad[:n] = x
    x_perm = x_pad[node_at_pos]
    b2_nonzero = bool(np.any(b2 != 0.0))
    for k in range(NCORES):
        s_k, d_k, n_s, n_d, self_k, key, counts = per_core[k]
        ne = s_k.shape[0]
        grp_off = np.zeros(NWG * NBLK + 1, np.int64)
        grp_off[1:] = np.cumsum(counts)
        rank = np.arange(ne, dtype=np.int64) - grp_off[key]
        pos = tile_base[key] * 128 + rank

        gidx = np.zeros(t_total * 128, np.int16)
        norm = np.zeros(t_total * 128, np.float32)
        dloc = np.zeros(t_total * 128, np.int64)
        # self edges index the local bounce (position % SHARD); real edges
        # index their 25600-row window of the gathered table.
        rel = np.where(self_k, s_k % SHARD, s_k - (s_k // WIN) * WIN)
        gidx[pos] = rel.astype(np.int16)
        norm[pos] = a[n_s] * a[n_d]
        dloc[pos] = d_k % 128

        # S tiles: [128 e, t, 128 d] flattened to [128, t_total*128]
        S = np.zeros((t_total, 128, 128), np.float32)
        tt = np.arange(t_total * 128) // 128
        ee = np.arange(t_total * 128) % 128
        S[tt, ee, dloc] = norm
        S_t = np.ascontiguousarray(S.transpose(1, 0, 2).reshape(128, t_total * 128))

        # wrapped gather indices per call, replicated across the 8 Q7 groups
        idxw = np.zeros((128, t_total * 8), np.int16)
        for (w, t0, nt) in calls:
            blk = gidx[t0 * 128:(t0 + nt) * 128].reshape(nt * 8, 16).T
            idxw[:, t0 * 8:(t0 + nt) * 8] = np.tile(blk, (8, 1))

        xT = np.ascontiguousarray(x_perm[k * SHARD:(k + 1) * SHARD].T)
        in_maps.append({
            "xT": xT,
            "S": S_t,
            "idxw": idxw,
            "W1": W1,
            "W2": W2,
            "b1col": b1.reshape(128, 1).copy(),
            "b2bc": np.broadcast_to(b2, (128, 128)).copy(),
        })

    sched_sig = (tuple(int(v) for v in T), tuple(calls), b2_nonzero)
    return in_maps, sched_sig, tuple(int(v) for v in tile_base), t_total, b2_nonzero, pos_of_node


def _build_program(tile_base, t_total, calls, b2_nonzero):
    nc = bacc.Bacc("TRN2", target_bir_lowering=False, debug=False,
                   num_devices=NCORES, num_swdge_queues=4)
    f32 = mybir.dt.float32
    xT_d = nc.dram_tensor("xT", [D, SHARD], f32, kind="ExternalInput")
    S_d = nc.dram_tensor("S", [128, t_total * 128], f32, kind="ExternalInput")
    idx_d = nc.dram_tensor("idxw", [128, t_total * 8], mybir.dt.int16, kind="ExternalInput")
    W1_d = nc.dram_tensor("W1", [D, D], f32, kind="ExternalInput")
    W2_d = nc.dram_tensor("W2", [D, D], f32, kind="ExternalInput")
    b1_d = nc.dram_tensor("b1col", [128, 1], f32, kind="ExternalInput")
    b2_d = nc.dram_tensor("b2bc", [128, 128], f32, kind="ExternalInput")
    out_d = nc.dram_tensor("out", [SHARD, D], f32, kind="ExternalOutput")

    h1_bounce = nc.dram_tensor("h1_bounce", [SHARD, D], f32)
    h1_full = nc.dram_tensor("h1_full", [NPAD, D], f32, addr_space="Shared")
    h2_bounce = nc.dram_tensor("h2_bounce", [SHARD, D], f32)
    h2_full = nc.dram_tensor("h2_full", [NPAD, D], f32, addr_space="Shared")

    with tile.TileContext(nc) as tc:
        with (
            tc.tile_pool(name="const", bufs=1) as p_const,
            tc.tile_pool(name="accbig", bufs=1) as p_acc,
            tc.tile_pool(name="msg", bufs=8) as p_msg,
            tc.tile_pool(name="sel", bufs=6) as p_sel,
            tc.tile_pool(name="small", bufs=3) as p_small,
            tc.tile_pool(name="dpsum", bufs=2, space="PSUM") as p_dpsum,
            tc.tile_pool(name="epsum", bufs=6, space="PSUM") as p_epsum,
        ):
            W1_t = p_const.tile([D, D], f32)
            W2_t = p_const.tile([D, D], f32)
            b1_t = p_const.tile([128, 1], f32)
            idx_t = p_const.tile([128, t_total * 8], mybir.dt.int16)
            nc.sync.dma_start(out=W1_t[:], in_=W1_d[:])
            nc.sync.dma_start(out=W2_t[:], in_=W2_d[:])
            nc.sync.dma_start(out=b1_t[:], in_=b1_d[:])
            nc.sync.dma_start(out=idx_t[:], in_=idx_d[:])
            if b2_nonzero:
                b2_t = p_const.tile([128, 128], f32)
                nc.sync.dma_start(out=b2_t[:], in_=b2_d[:])

            # ---------- dense 1: h1 = x @ W1 ----------
            with tc.tile_pool(name="xp", bufs=1) as p_x:
                xT_t = p_x.tile([D, SHARD], f32)
                nc.sync.dma_start(out=xT_t[:], in_=xT_d[:])
                for j in range(NBLK):
                    ps = p_dpsum.tile([128, D], f32, space="PSUM", tag="dps")
                    nc.tensor.matmul(out=ps[:], lhsT=xT_t[:, j * 128:(j + 1) * 128],
                                     rhs=W1_t[:], start=True, stop=True)
                    hb = p_small.tile([128, D], f32, tag="hsb")
                    nc.scalar.activation(out=hb[:], in_=ps[:],
                                         func=mybir.ActivationFunctionType.Copy)
                    nc.sync.dma_start(out=h1_bounce[j * 128:(j + 1) * 128, :], in_=hb[:])

            nc.gpsimd.collective_compute(
                "AllGather", mybir.AluOpType.bypass,
                replica_groups=[list(range(NCORES))],
                ins=[h1_bounce[:]], outs=[h1_full[:]],
            )

            acc1 = p_acc.tile([128, SHARD], f32, tag="acc")

            # ---------- edge phase ----------
            def edge_phase(h_bounce, h_full, acc, transposed):
                # iterate gather calls; matmul-accumulate per (w,b) group
                pending = {}
                for ci, (w, t0, nt) in enumerate(calls):
                    src_ap = (h_bounce[:] if w == 0
                              else h_full[(w - 1) * WIN:w * WIN, :])
                    msg_t = p_msg.tile([128, CHUNK_T, D], f32, tag="msg")
                    nc.gpsimd.dma_gather(
                        out_ap=msg_t[:, :nt, :],
                        in_ap=src_ap,
                        idxs_ap=idx_t[:, t0 * 8:(t0 + nt) * 8],
                        num_idxs=nt * 128, num_idxs_reg=nt * 128,
                        elem_size=D, queue_num=ci % 4)
                    S_t = p_sel.tile([128, CHUNK_T * 128], f32, tag="sel")
                    nc.sync.dma_start(out=S_t[:, :nt * 128],
                                      in_=S_d[:, t0 * 128:(t0 + nt) * 128])
                    # run matmuls for all tiles in this chunk
                    for t in range(t0, t0 + nt):
                        # which group is tile t in?
                        g = np.searchsorted(tile_base, t, side="right") - 1
                        gs, ge = tile_base[g], tile_base[g + 1]
                        first, last = (t == gs), (t == ge - 1)
                        b = g % NBLK
                        if first:
                            ps = p_epsum.tile([128, D], f32, space="PSUM", tag="eps")
                            pending[g] = ps
                        ps = pending[g]
                        mt = msg_t[:, t - t0, :]
                        st = S_t[:, (t - t0) * 128:(t - t0 + 1) * 128]
                        if transposed:
                            nc.tensor.matmul(out=ps[:], lhsT=mt, rhs=st,
                                             start=first, stop=last)
                        else:
                            nc.tensor.matmul(out=ps[:], lhsT=st, rhs=mt,
                                             start=first, stop=last)
                        if last:
                            dstsl = acc[:, b * 128:(b + 1) * 128]
                            if g < NBLK:  # window 0: initialize
                                nc.vector.tensor_copy(out=dstsl, in_=ps[:])
                            else:
                                nc.vector.tensor_add(out=dstsl, in0=dstsl, in1=ps[:])
                            del pending[g]

            edge_phase(h1_bounce, h1_full, acc1, transposed=True)

            # epilogue 1 (transposed world): out1T = relu(acc1 + b1)
            for b in range(NBLK):
                sl = acc1[:, b * 128:(b + 1) * 128]
                nc.scalar.activation(out=sl, in_=sl,
                                     func=mybir.ActivationFunctionType.Relu,
                                     bias=b1_t[:, :1])

            # ---------- dense 2: h2 = relu1 @ W2 ----------
            for j in range(NBLK):
                ps = p_dpsum.tile([128, D], f32, space="PSUM", tag="dps")
                nc.tensor.matmul(out=ps[:], lhsT=acc1[:, j * 128:(j + 1) * 128],
                                 rhs=W2_t[:], start=True, stop=True)
                hb = p_small.tile([128, D], f32, tag="hsb")
                nc.scalar.activation(out=hb[:], in_=ps[:],
                                     func=mybir.ActivationFunctionType.Copy)
                nc.sync.dma_start(out=h2_bounce[j * 128:(j + 1) * 128, :], in_=hb[:])

            nc.gpsimd.collective_compute(
                "AllGather", mybir.AluOpType.bypass,
                replica_groups=[list(range(NCORES))],
                ins=[h2_bounce[:]], outs=[h2_full[:]],
            )

            acc2 = p_acc.tile([128, SHARD], f32, tag="acc")
            edge_phase(h2_bounce, h2_full, acc2, transposed=False)

            # epilogue 2 (normal world): out = relu(acc2 [+ b2])
            for b in range(NBLK):
                sl = acc2[:, b * 128:(b + 1) * 128]
                if b2_nonzero:
                    nc.vector.tensor_add(out=sl, in0=sl, in1=b2_t[:])
                ob = p_small.tile([128, D], f32, tag="osb")
                nc.scalar.activation(out=ob[:], in_=sl,
                                     func=mybir.ActivationFunctionType.Relu)
                nc.sync.dma_start(out=out_d[b * 128:(b + 1) * 128, :], in_=ob[:])

    nc.compile()
    return nc


def prepare(x, edge_index, W1, b1, W2, b2):
    """Host prep + (cached) program build. Returns (nc, in_maps, pos_of_node)."""
    in_maps, sched_sig, tile_base, t_total, b2_nonzero, pos_of_node = _host_prep(
        x, edge_index, W1, b1, W2, b2)
    calls = sched_sig[1]
    key = sched_sig
    if key not in _CACHE:
        _CACHE[key] = _build_program(tile_base, t_total, list(calls), b2_nonzero)
    return _CACHE[key], in_maps, pos_of_node


def kernel(x, edge_index, W1, b1, W2, b2):
    nc, in_maps, pos_of_node = prepare(x, edge_index, W1, b1, W2, b2)
    res = run_bass_kernel_spmd(nc, in_maps, list(range(NCORES)))
    full = np.concatenate([res.results[k]["out"] for k in range(NCORES)], axis=0)
    n = np.asarray(x).shape[0]
    return full[pos_of_node[:n]]



# revision 2
# speedup vs baseline: 4.1998x; 4.1998x over previous
"""Two-layer GCN (PyG GCNConv x2 + ReLU) on 8 Trainium2 NeuronCores — v2.

Strategy (dst-sharded SPMD, fp16 data path, on-chip selection matrices):
  - Nodes padded to 102400, sharded 12800/core by destination via a
    degree-balanced permutation; 128x128 weights replicated.
  - Per layer: dense h = x_shard @ W (fp16 in, fp32 psum) -> quartered
    fp16 AllGather (window w = source-quarter w of every core's shard, so
    gathers can chase collective pieces) -> windowed dma_gather of h[src]
    (int16 idx) -> scatter-add via per-tile selection matrices
    S[e, dst] = (iota[d]==dloc[e]) * norm[e], built ON-CHIP with one DVE
    tensor_scalar per 128-edge tile (no S streaming from HBM) -> fp16
    128x128 matmuls accumulating per (window, superblock) into a [128,512]
    PSUM bank, added into an SBUF fp32 accumulator per superblock.
  - Self-loop contributions skip the gather: the dense output stays in
    SBUF and a diagonal S (same tensor_scalar trick) initializes the
    accumulator while layer-1's collective is still in flight.
  - Both layers accumulate transposed [f, dst]. Layer-1 epilogue
    relu(acc+b1) directly emits the fp16 lhsT for layer 2's dense matmul;
    layer-2 epilogue does relu(+b2), a PE transpose back to [node, f], and
    writes fp32 rows. Layer-2's collective pieces fire during layer-1's
    window-3 processing, hiding them behind the gather stream.
"""

import numpy as np

import concourse.bass as bass
import concourse.bacc as bacc
import concourse.mybir as mybir
import concourse.tile as tile
from concourse.bass_utils import run_bass_kernel_spmd

N = 100000
E = 640000
D = 128
NCORES = 8
NPAD = 102400
SHARD = NPAD // NCORES        # 12800
NBLK = SHARD // 128           # 100 dst blocks per core
SB = 4                        # dst blocks per superblock (one 2KB PSUM bank)
NSB = NBLK // SB              # 25 superblocks
NW = 4                        # gather windows == source quarters
QROWS = SHARD // NW           # 3200 bounce rows per collective piece
WIN = QROWS * NCORES          # 25600 rows per window table
CHUNK_T = 8                   # tiles per dma_gather call (1024 idx ring limit)
NGRP = NW * NBLK              # (w, s, bi) groups, window-major
NQ = 4                        # collective pieces per layer
JPQ = NBLK // NQ              # dense blocks per collective piece

f16 = mybir.dt.float16
f32 = mybir.dt.float32
i16 = mybir.dt.int16

# bisect flags (timing experiments; correctness breaks when skipping)
SKIP_CC = False
SKIP_GATHER = False
SKIP_EDGE = False

_CACHE = {}


def _host_prep(x, edge_index, W1, b1, W2, b2):
    x = np.asarray(x, dtype=np.float32)
    ei = np.asarray(edge_index)
    W1 = np.asarray(W1, dtype=np.float32)
    W2 = np.asarray(W2, dtype=np.float32)
    b1 = np.asarray(b1, dtype=np.float32)
    b2 = np.asarray(b2, dtype=np.float32)
    n = x.shape[0]

    src = ei[0].astype(np.int64)
    dst = ei[1].astype(np.int64)
    deg = np.bincount(np.concatenate([dst, np.arange(n, dtype=np.int64)]),
                      minlength=NPAD).astype(np.float32)
    a = np.zeros(NPAD, np.float32)
    nz = deg > 0
    a[nz] = 1.0 / np.sqrt(deg[nz])

    # degree-balanced node->position permutation
    order_by_deg = np.argsort(-deg, kind="stable")
    i = np.arange(NPAD, dtype=np.int64)
    cb = i % (NCORES * NBLK)
    position_of_rank = (cb % NCORES) * SHARD + (cb // NCORES) * 128 + i // (NCORES * NBLK)
    pos_of_node = np.empty(NPAD, np.int64)
    pos_of_node[order_by_deg] = position_of_rank
    node_at_pos = np.empty(NPAD, np.int64)
    node_at_pos[pos_of_node] = np.arange(NPAD, dtype=np.int64)

    ps = pos_of_node[src]
    pd = pos_of_node[dst]
    core = pd // SHARD
    # window = source quarter (AllGather piece q = rows [q*QROWS,(q+1)*QROWS)
    # of every core's shard, concatenated by core)
    w_e = (ps % SHARD) // QROWS
    row_e = (ps // SHARD) * QROWS + (ps % SHARD) - w_e * QROWS
    b_e = (pd % SHARD) // 128
    g_e = w_e * NBLK + b_e            # window-major group id
    dloc_e = pd % 128
    norm_e = a[src] * a[dst]

    per_core = []
    counts_all = np.zeros((NCORES, NGRP), np.int64)
    for k in range(NCORES):
        m = core == k
        g_k, row_k, dloc_k, norm_k = g_e[m], row_e[m], dloc_e[m], norm_e[m]
        order = np.lexsort((row_k, g_k))
        g_k, row_k, dloc_k, norm_k = (g_k[order], row_k[order],
                                      dloc_k[order], norm_k[order])
        counts_all[k] = np.bincount(g_k, minlength=NGRP)
        per_core.append((g_k, row_k, dloc_k, norm_k))

    # every group gets >=1 tile so each PSUM slice is initialized (pad tiles
    # carry norm=0 and are harmless)
    T = np.maximum((np.max(counts_all, axis=0) + 127) // 128, 1)
    tile_base = np.zeros(NGRP + 1, np.int64)
    tile_base[1:] = np.cumsum(T)
    t_total = int(tile_base[-1])

    # tile -> (block-in-superblock, start, stop) and per-(w,s) tile ranges
    tinfo = []
    for g in range(NGRP):
        bi = g % SB
        for t in range(int(tile_base[g]), int(tile_base[g + 1])):
            tinfo.append((bi, t == int(tile_base[g]),
                          t == int(tile_base[g + 1]) - 1))
    ws_range = {}
    for w in range(NW):
        for s in range(NSB):
            g0 = w * NBLK + s * SB
            ws_range[(w, s)] = (int(tile_base[g0]), int(tile_base[g0 + SB]))
    calls = []  # (w, s_of_call_start, t0, nt): chunks within a window
    for w in range(NW):
        for s in range(NSB):
            t0, t_end = ws_range[(w, s)]
            t = t0
            while t < t_end:
                nt = min(CHUNK_T, t_end - t)
                calls.append((w, s, t, nt))
                t += nt

    x_pad = np.zeros((NPAD, D), np.float32)
    x_pad[:n] = x
    x_perm = x_pad[node_at_pos]
    a2_pos = (a[node_at_pos] ** 2).astype(np.float32)

    iota32 = np.tile(np.arange(128, dtype=np.float32), (128, 1))
    ident16 = np.eye(128, dtype=np.float16)
    dcol32 = np.arange(128, dtype=np.float32).reshape(128, 1)

    in_maps = []
    for k in range(NCORES):
        g_k, row_k, dloc_k, norm_k = per_core[k]
        ne = g_k.shape[0]
        grp_off = np.zeros(NGRP + 1, np.int64)
        grp_off[1:] = np.cumsum(counts_all[k])
        rank = np.arange(ne, dtype=np.int64) - grp_off[g_k]
        slot = tile_base[g_k] * 128 + rank

        gidx = np.zeros(t_total * 128, np.int16)
        gidx[slot] = row_k.astype(np.int16)
        dlocA = np.zeros(t_total * 128, np.float32)
        dlocA[slot] = dloc_k.astype(np.float32)
        normA = np.zeros(t_total * 128, np.float32)
        normA[slot] = norm_k
        dloc32 = np.ascontiguousarray(dlocA.reshape(t_total, 128).T)
        norm32 = np.ascontiguousarray(normA.reshape(t_total, 128).T)

        idxw = np.zeros((128, t_total * 8), np.int16)
        for (w, s, t0, nt) in calls:
            blk = gidx[t0 * 128:(t0 + nt) * 128].reshape(nt * 8, 16).T
            idxw[:, t0 * 8:(t0 + nt) * 8] = np.tile(blk, (8, 1))

        snorm32 = np.ascontiguousarray(
            a2_pos[k * SHARD:(k + 1) * SHARD].reshape(NBLK, 128).T)
        xT16 = np.ascontiguousarray(
            x_perm[k * SHARD:(k + 1) * SHARD].T).astype(np.float16)

        in_maps.append({
            "xT16": xT16,
            "W1_16": W1.astype(np.float16),
            "W2_16": W2.astype(np.float16),
            "b1col": b1.reshape(128, 1).copy(),
            "b2col": b2.reshape(128, 1).copy(),
            "iota32": iota32,
            "ident16": ident16,
            "dcol32": dcol32,
            "snorm32": snorm32,
            "dloc32": dloc32,
            "norm32": norm32,
            "idxw": idxw,
        })

    sched_sig = tuple(int(v) for v in T)
    return (in_maps, sched_sig, tuple(int(v) for v in tile_base), t_total,
            tinfo, ws_range, calls, pos_of_node)


def _build_program(tile_base, t_total, tinfo, ws_range, calls):
    nc = bacc.Bacc("TRN2", target_bir_lowering=False, debug=False,
                   num_devices=NCORES, num_swdge_queues=4)
    xT_d = nc.dram_tensor("xT16", [D, SHARD], f16, kind="ExternalInput")
    W1_d = nc.dram_tensor("W1_16", [D, D], f16, kind="ExternalInput")
    W2_d = nc.dram_tensor("W2_16", [D, D], f16, kind="ExternalInput")
    b1_d = nc.dram_tensor("b1col", [128, 1], f32, kind="ExternalInput")
    b2_d = nc.dram_tensor("b2col", [128, 1], f32, kind="ExternalInput")
    iota_d = nc.dram_tensor("iota32", [128, 128], f32, kind="ExternalInput")
    ident_d = nc.dram_tensor("ident16", [128, 128], f16, kind="ExternalInput")
    dcol_d = nc.dram_tensor("dcol32", [128, 1], f32, kind="ExternalInput")
    snorm_d = nc.dram_tensor("snorm32", [128, NBLK], f32, kind="ExternalInput")
    dloc_d = nc.dram_tensor("dloc32", [128, t_total], f32, kind="ExternalInput")
    norm_d = nc.dram_tensor("norm32", [128, t_total], f32, kind="ExternalInput")
    idx_d = nc.dram_tensor("idxw", [128, t_total * 8], i16, kind="ExternalInput")
    out_d = nc.dram_tensor("out", [SHARD, D], f32, kind="ExternalOutput")

    h_bq = [[nc.dram_tensor(f"h{l}_bq{q}", [QROWS, D], f16) for q in range(NQ)]
            for l in range(2)]
    h_w = [[nc.dram_tensor(f"h{l}_w{q}", [WIN, D], f16, addr_space="Shared")
            for q in range(NQ)] for l in range(2)]

    with tile.TileContext(nc) as tc:
        with (
            tc.tile_pool(name="const", bufs=1) as p_const,
            tc.tile_pool(name="big", bufs=1) as p_big,
            tc.tile_pool(name="msg", bufs=12) as p_msg,
            tc.tile_pool(name="sel", bufs=16) as p_sel,
            tc.tile_pool(name="r16", bufs=2) as p_r16,
            tc.tile_pool(name="o32", bufs=4) as p_o32,
            tc.tile_pool(name="wps", bufs=3, space="PSUM") as p_wps,
            tc.tile_pool(name="dps", bufs=2, space="PSUM") as p_dps,
            tc.tile_pool(name="tps", bufs=2, space="PSUM") as p_tps,
        ):
            W1_t = p_const.tile([D, D], f16)
            W2_t = p_const.tile([D, D], f16)
            b1_t = p_const.tile([128, 1], f32)
            b2_t = p_const.tile([128, 1], f32)
            iota_t = p_const.tile([128, 128], f32)
            ident_t = p_const.tile([128, 128], f16)
            dcol_t = p_const.tile([128, 1], f32)
            snorm_t = p_const.tile([128, NBLK], f32)
            dloc_t = p_const.tile([128, t_total], f32)
            norm_t = p_const.tile([128, t_total], f32)
            idx_t = p_const.tile([128, t_total * 8], i16)
            xT_t = p_const.tile([D, SHARD], f16)
            relu1_t = p_big.tile([128, SHARD], f16, tag="relu1")
            for tt, dd in ((W1_t, W1_d), (W2_t, W2_d), (b1_t, b1_d),
                           (b2_t, b2_d), (iota_t, iota_d), (ident_t, ident_d),
                           (dcol_t, dcol_d), (snorm_t, snorm_d),
                           (dloc_t, dloc_d), (norm_t, norm_d), (idx_t, idx_d),
                           (xT_t, xT_d)):
                nc.sync.dma_start(out=tt[:], in_=dd[:])

            def build_S(out_ap, scalar1, scalar2):
                nc.vector.tensor_scalar(
                    out=out_ap, in0=iota_t[:], scalar1=scalar1, scalar2=scalar2,
                    op0=mybir.AluOpType.is_equal, op1=mybir.AluOpType.mult)

            def dense_block(lhsT_full, W_t, hloc_t, l, j):
                ps = p_dps.tile([128, D], f32, space="PSUM", tag="dps")
                nc.tensor.matmul(out=ps[:],
                                 lhsT=lhsT_full[:, j * 128:(j + 1) * 128],
                                 rhs=W_t[:], start=True, stop=True)
                hsl = hloc_t[:, j * 128:(j + 1) * 128]
                nc.scalar.activation(out=hsl, in_=ps[:],
                                     func=mybir.ActivationFunctionType.Copy)
                q, jr = j // JPQ, j % JPQ
                nc.sync.dma_start(out=h_bq[l][q][jr * 128:(jr + 1) * 128, :],
                                  in_=hsl)
                if (j + 1) % JPQ == 0 and not SKIP_CC:
                    nc.gpsimd.collective_compute(
                        "AllGather", mybir.AluOpType.bypass,
                        replica_groups=[list(range(NCORES))],
                        ins=[h_bq[l][q][:]], outs=[h_w[l][q][:]])

            msg0 = None
            if SKIP_GATHER:
                msg0 = p_const.tile([128, CHUNK_T, D], f16)
                nc.vector.memset(msg0[:], 0.5)

            def edge_phase(l, hloc_t, acc, epilogue_cb):
                # self phase: diagonal S from the SBUF dense output
                for s in range(NSB):
                    pw = p_wps.tile([128, SB * 128], f32, space="PSUM",
                                    tag="wps")
                    for bi in range(SB):
                        b = s * SB + bi
                        Ssf = p_sel.tile([128, 128], f16, tag="sel")
                        build_S(Ssf[:], dcol_t[:, :1], snorm_t[:, b:b + 1])
                        nc.tensor.matmul(
                            out=pw[:, bi * 128:(bi + 1) * 128],
                            lhsT=hloc_t[:, b * 128:(b + 1) * 128], rhs=Ssf[:],
                            start=True, stop=True, skip_group_check=True)
                    nc.scalar.activation(
                        out=acc[:, s * SB * 128:(s + 1) * SB * 128],
                        in_=pw[:], func=mybir.ActivationFunctionType.Copy)
                # window phases, chasing the collective pieces
                call_i = 0
                for w in range(NW):
                    s_open = -1
                    pw = None
                    for s in range(NSB):
                        t0, t_end = ws_range[(w, s)]
                        if t0 == t_end:
                            if w == NW - 1:
                                epilogue_cb(s)
                            continue
                        while call_i < len(calls) and calls[call_i][2] < t_end \
                                and calls[call_i][0] == w:
                            _, _, c0, cnt = calls[call_i]
                            if SKIP_GATHER:
                                msg = msg0
                            else:
                              msg = p_msg.tile([128, CHUNK_T, D], f16, tag="msg")
                              nc.gpsimd.dma_gather(
                                out_ap=msg[:, :cnt, :], in_ap=h_w[l][w][:],
                                idxs_ap=idx_t[:, c0 * 8:(c0 + cnt) * 8],
                                num_idxs=cnt * 128, num_idxs_reg=cnt * 128,
                                elem_size=D, queue_num=call_i % 4)
                            call_i += 1
                            for t in range(c0, c0 + cnt):
                                bi_t, start_t, stop_t = tinfo[t]
                                if SKIP_EDGE:
                                    if t == t_end - 1 and w == NW - 1:
                                        epilogue_cb(s)
                                    continue
                                if t == t0:
                                    pw = p_wps.tile([128, SB * 128], f32,
                                                    space="PSUM", tag="wps")
                                    s_open = s
                                St = p_sel.tile([128, 128], f16, tag="sel")
                                build_S(St[:], dloc_t[:, t:t + 1],
                                        norm_t[:, t:t + 1])
                                nc.tensor.matmul(
                                    out=pw[:, bi_t * 128:(bi_t + 1) * 128],
                                    lhsT=msg[:, t - c0, :], rhs=St[:],
                                    start=start_t, stop=stop_t,
                                    skip_group_check=True)
                                if t == t_end - 1:
                                    sl = acc[:, s * SB * 128:(s + 1) * SB * 128]
                                    nc.vector.tensor_add(out=sl, in0=sl,
                                                         in1=pw[:])
                                    if w == NW - 1:
                                        epilogue_cb(s)
                        # calls list is window-major so the inner while covers
                        # every superblock of this window in order

            # ---------- layer 1 ----------
            hloc_t = p_big.tile([128, SHARD], f16, tag="hloc")
            for j in range(NBLK):
                dense_block(xT_t, W1_t, hloc_t, 0, j)

            acc1 = p_big.tile([128, SHARD], f32, tag="acc")
            hloc2_t = None

            def epi1(s):
                nonlocal hloc2_t
                nc.scalar.activation(
                    out=relu1_t[:, s * 512:(s + 1) * 512],
                    in_=acc1[:, s * 512:(s + 1) * 512],
                    func=mybir.ActivationFunctionType.Relu, bias=b1_t[:, :1])
                for bi in range(SB):
                    b = s * SB + bi
                    dense_block(relu1_t, W2_t, hloc2_t, 1, b)

            hloc2_t = p_big.tile([128, SHARD], f16, tag="hloc")
            edge_phase(0, hloc_t, acc1, epi1)

            # ---------- layer 2 ----------
            acc2 = p_big.tile([128, SHARD], f32, tag="acc")

            def epi2(s):
                r16 = p_r16.tile([128, 512], f16, tag="r16")
                nc.scalar.activation(
                    out=r16[:], in_=acc2[:, s * 512:(s + 1) * 512],
                    func=mybir.ActivationFunctionType.Relu, bias=b2_t[:, :1])
                for bi in range(SB):
                    b = s * SB + bi
                    tp = p_tps.tile([128, 128], f16, space="PSUM", tag="tps")
                    nc.tensor.transpose(tp[:], r16[:, bi * 128:(bi + 1) * 128],
                                        ident_t[:])
                    o32 = p_o32.tile([128, 128], f32, tag="o32")
                    nc.scalar.activation(out=o32[:], in_=tp[:],
                                         func=mybir.ActivationFunctionType.Copy)
                    nc.sync.dma_start(out=out_d[b * 128:(b + 1) * 128, :],
                                      in_=o32[:])

            edge_phase(1, hloc2_t, acc2, epi2)

    nc.compile()
    return nc


def prepare(x, edge_index, W1, b1, W2, b2):
    (in_maps, sched_sig, tile_base, t_total, tinfo, ws_range, calls,
     pos_of_node) = _host_prep(x, edge_index, W1, b1, W2, b2)
    key = (sched_sig, SKIP_CC, SKIP_GATHER, SKIP_EDGE)
    if key not in _CACHE:
        _CACHE[key] = _build_program(tile_base, t_total, tinfo, ws_range,
                                     calls)
    return _CACHE[key], in_maps, pos_of_node


def kernel(x, edge_index, W1, b1, W2, b2):
    nc, in_maps, pos_of_node = prepare(x, edge_index, W1, b1, W2, b2)
    res = run_bass_kernel_spmd(nc, in_maps, list(range(NCORES)))
    full = np.concatenate([res.results[k]["out"] for k in range(NCORES)], axis=0)
    n = np.asarray(x).shape[0]
    return full[pos_of_node[:n]]
